# revision 10
# baseline (speedup 1.0000x reference)
"""BitLinearPacked kernel for Trainium2 (8 NeuronCores, data-parallel).

y = x @ w.T where w = unpack_sign_bits(packed) in {-1, +1}.
  x: [2, 8192, 1024] fp32, packed: [1024, 128] int32 (8 sign bits / byte,
  MSB-first within each byte).

Strategy
--------
Data-parallel over the 16384 flattened rows of x: each of the 8 cores gets
2048 rows; the packed weight (128 KB as uint8) is replicated.

On-chip, matmul contracts over the partition dim, so both operands need
in_features (k) on partitions. We pre-transpose each x shard on the host
into [1024, 2048] — and, crucially, permute k as k' = b*128 + j (b = bit
index, j = byte index, k = 8j + b). Under that permutation the weight
plane for bit b is exactly ((packed.T >> (7-b)) & 1) * 2 - 1, computed
lane-local from one [128, 1024] uint8 tile of packed.T — no partition
remap / weight transpose needed on chip. The contraction is permutation-
invariant, so y is unchanged and comes out in natural [rows, out] layout.

Matmul dtype: fp16 streams 1 cycle/row through the PE (vs 2 for f32r and
4 for fp32); the weight (+/-1) is exact in any dtype, so the only error
is rounding x to fp16 (~11-bit significand) -> ~2e-4 absmax-relative.

Latency engineering (the steady-state MM stream is already at roofline):
- packed is shipped as uint8 (128 KB) and loaded via the scalar-engine
  HWDGE queue so it doesn't queue behind x loads on sync.
- window 0 runs bit-plane-major in two half-column phases: per plane, a
  small x DMA + half-column unpack + 4 matmuls, accumulating in 4 live
  PSUM banks per phase — the PE starts ~3 us after the preamble instead
  of waiting for the full unpack + full window DMA.
- dummy matmuls on a zeroed tile fill the initial DMA-wait so the PE's
  HAM clock gate is already at 2.4 GHz when the real stream starts.
- y stores go out on the scalar HWDGE queue, keeping sync free for x.
"""

import numpy as np

import concourse.bass as bass
import concourse.tile as tile
from concourse import bacc, mybir
from concourse.bass_utils import run_bass_kernel_spmd

NCORES = 8
R = 2048   # rows per core (16384 / 8)
K = 1024   # in_features
O = 1024   # out_features
RW = 512   # row window per x DMA

MM_DTYPE = "fp16"  # "fp16" | "bf16" | "f32r"
_DT = {
    "fp16": mybir.dt.float16,
    "bf16": mybir.dt.bfloat16,
    "f32r": mybir.dt.float32r,
}
_NP_DT = {"fp16": np.float16, "f32r": np.float32}
N_WARMUP_MM = 7


def _build_nc(mm_dtype: str = MM_DTYPE) -> bass.Bass:
    dt = _DT[mm_dtype]
    nc = bacc.Bacc("TRN2", target_bir_lowering=False, debug=False)
    xp = nc.declare_dram_parameter("xp", [K, R], dt, isOutput=False)
    pkt = nc.declare_dram_parameter("pkt", [128, O], mybir.dt.uint8, isOutput=False)
    xs_d = nc.declare_dram_parameter("xs", [R], mybir.dt.float32, isOutput=False)
    y = nc.declare_dram_parameter("y", [R, O], mybir.dt.float32, isOutput=True)

    # [K, R] -> [128 partitions, 8 k-chunks, R]
    xp_v = xp.rearrange("(c p) r -> p c r", p=128)
    xs_v = xs_d.rearrange("(t p) -> p t", p=128)
    n_oc = O // 512
    n_rt = RW // 128

    with tile.TileContext(nc) as tc:
        with (
            tc.tile_pool(name="wpool", bufs=1) as wpool,
            tc.tile_pool(name="pkpool", bufs=1) as pkpool,
            tc.tile_pool(name="xpool", bufs=2) as xpool,
            tc.tile_pool(name="ypool", bufs=3) as ypool,
            tc.tile_pool(name="pspool", bufs=7, space="PSUM") as pspool,
            tc.tile_pool(name="warmps", bufs=1, space="PSUM") as warmps,
        ):
            # --- PE warm-up: dummy matmuls on a zeroed tile, no data deps ---
            warm_sb = wpool.tile([128, 512], dt, name="warm_sb")
            nc.vector.memset(warm_sb[:], 0.0)
            warm_ps = warmps.tile([128, 512], mybir.dt.float32, name="warm_ps")
            for i in range(N_WARMUP_MM):
                nc.tensor.matmul(
                    warm_ps[:], lhsT=warm_sb[:, :128], rhs=warm_sb[:],
                    start=True, stop=True,
                )

            pk_t = pkpool.tile([128, O], mybir.dt.uint8)
            nc.scalar.dma_start(pk_t[:], pkt[:])
            xs_t = pkpool.tile([128, R // 128], mybir.dt.float32, name="xs_t")
            nc.scalar.dma_start(xs_t[:], xs_v[:])

            w_t = wpool.tile([128, 8, O], dt)
            x0_t = xpool.tile([128, 8, RW], dt, name="x0_t")


            # --- window 0: bit-plane-major, two half-column phases ---
            ps0 = [
                pspool.tile([128, 512], mybir.dt.float32, name=f"ps0_{i}", tag="ps")
                for i in range(n_rt * n_oc)
            ]
            for b in range(8):
                nc.sync.dma_start(x0_t[:, b:b + 1, :], xp_v[:, b:b + 1, 0:RW])
            for oc in range(n_oc):
                cs = slice(oc * 512, (oc + 1) * 512)
                for b in range(8):
                    bits = pkpool.tile(
                        [128, 512], mybir.dt.uint8, name=f"bits_{oc}_{b}", tag="bits"
                    )
                    nc.vector.tensor_scalar(
                        bits[:], pk_t[:, cs], 7 - b, 1,
                        mybir.AluOpType.logical_shift_right,
                        mybir.AluOpType.bitwise_and,
                    )
                    nc.scalar.copy(w_t[:, b, cs], bits[:])
                    for rt in range(n_rt):
                        nc.tensor.matmul(
                            ps0[rt * n_oc + oc][:],
                            lhsT=x0_t[:, b, rt * 128:(rt + 1) * 128],
                            rhs=w_t[:, b, cs],
                            start=(b == 0),
                            stop=(b == 7),
                        )
            for rt in range(n_rt):
                y_t = ypool.tile([128, O], mybir.dt.float32, name=f"y0_{rt}", tag="y_t")
                for oc in range(n_oc):
                    nc.vector.tensor_scalar(
                        y_t[:, oc * 512:(oc + 1) * 512], ps0[rt * n_oc + oc][:],
                        2.0, xs_t[:, rt:rt + 1],
                        mybir.AluOpType.mult, mybir.AluOpType.subtract,
                    )
                nc.scalar.dma_start(y[rt * 128:(rt + 1) * 128, :], y_t[:])

            # --- steady state: row-tile-major ---
            for rw in range(1, R // RW):
                x_t = xpool.tile([128, 8, RW], dt, name=f"x_t{rw}", tag="x_t")
                nc.sync.dma_start(x_t[:], xp_v[:, :, rw * RW:(rw + 1) * RW])
                for rt in range(n_rt):
                    r0 = rw * RW + rt * 128
                    y_t = ypool.tile(
                        [128, O], mybir.dt.float32, name=f"y_{rw}_{rt}", tag="y_t"
                    )
                    last_tile = (rw == R // RW - 1) and (rt == n_rt - 1)
                    for oc in range(n_oc):
                        ps = pspool.tile(
                            [128, 512], mybir.dt.float32,
                            name=f"ps_{rw}_{rt}_{oc}", tag="ps",
                        )
                        for b in range(8):
                            nc.tensor.matmul(
                                ps[:],
                                lhsT=x_t[:, b, rt * 128:(rt + 1) * 128],
                                rhs=w_t[:, b, oc * 512:(oc + 1) * 512],
                                start=(b == 0),
                                stop=(b == 7),
                            )
                        nc.vector.tensor_scalar(
                            y_t[:, oc * 512:(oc + 1) * 512], ps[:],
                            2.0, xs_t[:, rw * n_rt + rt:rw * n_rt + rt + 1],
                            mybir.AluOpType.mult, mybir.AluOpType.subtract,
                        )
                        if last_tile:
                            eng = nc.scalar if oc == 0 else nc.sync
                            eng.dma_start(
                                y[r0:r0 + 128, oc * 512:(oc + 1) * 512],
                                y_t[:, oc * 512:(oc + 1) * 512],
                            )
                    if not last_tile:
                        nc.scalar.dma_start(y[r0:r0 + 128, :], y_t[:])
    nc.finalize()
    return nc


_NC_CACHE = {}


def _get_nc(mm_dtype: str = MM_DTYPE):
    if mm_dtype not in _NC_CACHE:
        _NC_CACHE[mm_dtype] = _build_nc(mm_dtype)
    return _NC_CACHE[mm_dtype]


def _make_in_maps(x: np.ndarray, packed: np.ndarray, mm_dtype: str = MM_DTYPE):
    import ml_dtypes

    np_dt = _NP_DT.get(mm_dtype, np.dtype(ml_dtypes.bfloat16))
    xf = np.ascontiguousarray(x, dtype=np.float32).reshape(NCORES * R, K)
    pkt = np.ascontiguousarray(packed.T.astype(np.uint8))  # [128, 1024]
    in_maps = []
    for c in range(NCORES):
        xs = xf[c * R:(c + 1) * R]                       # [R, K]
        # k = 8j + b  ->  k' = b*128 + j ; [R,K]->[R,128,8]->[8,128,R]->[K,R]
        xp = np.ascontiguousarray(
            xs.reshape(R, 128, 8).transpose(2, 1, 0), dtype=np_dt
        ).reshape(K, R)
        # y = 2*(x @ bits.T) - rowsum(x): rowsum of the fp16-rounded shard
        srow = xp.astype(np.float64).sum(axis=0).astype(np.float32)  # [R]
        in_maps.append({"xp": xp, "pkt": pkt, "xs": srow})
    return in_maps


def kernel(x: np.ndarray, packed: np.ndarray) -> np.ndarray:
    x = np.asarray(x)
    packed = np.asarray(packed)
    assert x.shape == (2, 8192, K) and packed.shape == (O, K // 8)

    in_maps = _make_in_maps(x, packed)
    nc = _get_nc()
    res = run_bass_kernel_spmd(nc, in_maps, core_ids=list(range(NCORES)))
    out = np.concatenate([res.results[c]["y"] for c in range(NCORES)], axis=0)
    return out.reshape(2, 8192, O).astype(np.float32, copy=False)


# revision 11
# speedup vs baseline: 1.0369x; 1.0369x over previous
"""BitLinearPacked kernel for Trainium2 (8 NeuronCores, data-parallel).

y = x @ w.T where w = unpack_sign_bits(packed) in {-1, +1}.
  x: [2, 8192, 1024] fp32, packed: [1024, 128] int32 (8 sign bits / byte,
  MSB-first within each byte).

Strategy
--------
Data-parallel over the 16384 flattened rows of x: each of the 8 cores gets
2048 rows; the packed weight (128 KB as uint8) is replicated.

On-chip, matmul contracts over the partition dim, so both operands need
in_features (k) on partitions. We pre-transpose each x shard on the host
into [1024, 2048] — and, crucially, permute k as k' = b*128 + j (b = bit
index, j = byte index, k = 8j + b). Under that permutation the weight
plane for bit b is exactly ((packed.T >> (7-b)) & 1) * 2 - 1, computed
lane-local from one [128, 1024] uint8 tile of packed.T — no partition
remap / weight transpose needed on chip. The contraction is permutation-
invariant, so y is unchanged and comes out in natural [rows, out] layout.

Matmul dtype: fp16 streams 1 cycle/row through the PE (vs 2 for f32r and
4 for fp32); the weight (+/-1) is exact in any dtype, so the only error
is rounding x to fp16 (~11-bit significand) -> ~2e-4 absmax-relative.

Latency engineering (the steady-state MM stream is already at roofline):
- packed is shipped as uint8 (128 KB) and loaded via the scalar-engine
  HWDGE queue so it doesn't queue behind x loads on sync.
- window 0 runs bit-plane-major in two half-column phases: per plane, a
  small x DMA + half-column unpack + 4 matmuls, accumulating in 4 live
  PSUM banks per phase — the PE starts ~3 us after the preamble instead
  of waiting for the full unpack + full window DMA.
- dummy matmuls on a zeroed tile fill the initial DMA-wait so the PE's
  HAM clock gate is already at 2.4 GHz when the real stream starts.
- y stores go out on the scalar HWDGE queue, keeping sync free for x.
"""

import numpy as np

import concourse.bass as bass
import concourse.tile as tile
from concourse import bacc, mybir
from concourse.bass_utils import run_bass_kernel_spmd

NCORES = 8
R = 2048   # rows per core (16384 / 8)
K = 1024   # in_features
O = 1024   # out_features
RW = 512   # row window per x DMA

MM_DTYPE = "fp16"  # "fp16" | "bf16" | "f32r"
_DT = {
    "fp16": mybir.dt.float16,
    "bf16": mybir.dt.bfloat16,
    "f32r": mybir.dt.float32r,
}
_NP_DT = {"fp16": np.float16, "f32r": np.float32}
N_WARMUP_MM = 7


def _build_nc(mm_dtype: str = MM_DTYPE) -> bass.Bass:
    dt = _DT[mm_dtype]
    nc = bacc.Bacc("TRN2", target_bir_lowering=False, debug=False)
    xp = nc.declare_dram_parameter("xp", [K, R], dt, isOutput=False)
    pkt = nc.declare_dram_parameter("pkt", [128, O], mybir.dt.uint8, isOutput=False)
    xs_d = nc.declare_dram_parameter("xs", [R], mybir.dt.float32, isOutput=False)
    y = nc.declare_dram_parameter("y", [R, O], mybir.dt.float32, isOutput=True)

    # [K, R] -> [128 partitions, 8 k-chunks, R]
    xp_v = xp.rearrange("(c p) r -> p c r", p=128)
    xs_v = xs_d.rearrange("(t p) -> p t", p=128)
    n_oc = O // 512
    n_rt = RW // 128

    with tile.TileContext(nc) as tc:
        with (
            tc.tile_pool(name="wpool", bufs=1) as wpool,
            tc.tile_pool(name="pkpool", bufs=1) as pkpool,
            tc.tile_pool(name="bitpool", bufs=4) as bitpool,
            tc.tile_pool(name="xpool", bufs=2) as xpool,
            tc.tile_pool(name="ypool", bufs=3) as ypool,
            tc.tile_pool(name="pspool", bufs=7, space="PSUM") as pspool,
            tc.tile_pool(name="warmps", bufs=1, space="PSUM") as warmps,
        ):
            # --- PE warm-up: dummy matmuls on a zeroed tile, no data deps ---
            warm_sb = wpool.tile([128, 512], dt, name="warm_sb")
            nc.vector.memset(warm_sb[:], 0.0)
            warm_ps = warmps.tile([128, 512], mybir.dt.float32, name="warm_ps")
            for i in range(N_WARMUP_MM):
                nc.tensor.matmul(
                    warm_ps[:], lhsT=warm_sb[:, :128], rhs=warm_sb[:],
                    start=True, stop=True,
                )

            pk_t = pkpool.tile([128, O], mybir.dt.uint8)
            nc.scalar.dma_start(pk_t[:], pkt[:])
            xs_t = pkpool.tile([128, R // 128], mybir.dt.float32, name="xs_t")
            nc.scalar.dma_start(xs_t[:], xs_v[:])

            w_t = wpool.tile([128, 8, O], dt)
            x0_t = xpool.tile([128, 8, RW], dt, name="x0_t")


            # --- window 0: bit-plane-major, two half-column phases ---
            ps0 = [
                pspool.tile([128, 512], mybir.dt.float32, name=f"ps0_{i}", tag="ps")
                for i in range(n_rt * n_oc)
            ]
            for b in range(8):
                nc.sync.dma_start(x0_t[:, b:b + 1, :], xp_v[:, b:b + 1, 0:RW])
            for oc in range(n_oc):
                cs = slice(oc * 512, (oc + 1) * 512)
                for b in range(8):
                    bits = bitpool.tile(
                        [128, 512], mybir.dt.uint8, name=f"bits_{oc}_{b}", tag="bits"
                    )
                    nc.vector.tensor_scalar(
                        bits[:], pk_t[:, cs], 7 - b, 1,
                        mybir.AluOpType.logical_shift_right,
                        mybir.AluOpType.bitwise_and,
                    )
                    nc.scalar.copy(w_t[:, b, cs], bits[:])
                    for rt in range(n_rt):
                        nc.tensor.matmul(
                            ps0[rt * n_oc + oc][:],
                            lhsT=x0_t[:, b, rt * 128:(rt + 1) * 128],
                            rhs=w_t[:, b, cs],
                            start=(b == 0),
                            stop=(b == 7),
                        )
            for rt in range(n_rt):
                y_t = ypool.tile([128, O], mybir.dt.float32, name=f"y0_{rt}", tag="y_t")
                for oc in range(n_oc):
                    nc.vector.tensor_scalar(
                        y_t[:, oc * 512:(oc + 1) * 512], ps0[rt * n_oc + oc][:],
                        2.0, xs_t[:, rt:rt + 1],
                        mybir.AluOpType.mult, mybir.AluOpType.subtract,
                    )
                nc.scalar.dma_start(y[rt * 128:(rt + 1) * 128, :], y_t[:])

            # --- steady state: row-tile-major ---
            for rw in range(1, R // RW):
                x_t = xpool.tile([128, 8, RW], dt, name=f"x_t{rw}", tag="x_t")
                nc.sync.dma_start(x_t[:], xp_v[:, :, rw * RW:(rw + 1) * RW])
                for rt in range(n_rt):
                    r0 = rw * RW + rt * 128
                    y_t = ypool.tile(
                        [128, O], mybir.dt.float32, name=f"y_{rw}_{rt}", tag="y_t"
                    )
                    last_tile = (rw == R // RW - 1) and (rt == n_rt - 1)
                    for oc in range(n_oc):
                        ps = pspool.tile(
                            [128, 512], mybir.dt.float32,
                            name=f"ps_{rw}_{rt}_{oc}", tag="ps",
                        )
                        for b in range(8):
                            nc.tensor.matmul(
                                ps[:],
                                lhsT=x_t[:, b, rt * 128:(rt + 1) * 128],
                                rhs=w_t[:, b, oc * 512:(oc + 1) * 512],
                                start=(b == 0),
                                stop=(b == 7),
                            )
                        nc.vector.tensor_scalar(
                            y_t[:, oc * 512:(oc + 1) * 512], ps[:],
                            2.0, xs_t[:, rw * n_rt + rt:rw * n_rt + rt + 1],
                            mybir.AluOpType.mult, mybir.AluOpType.subtract,
                        )
                        if last_tile:
                            eng = nc.scalar if oc == 0 else nc.sync
                            eng.dma_start(
                                y[r0:r0 + 128, oc * 512:(oc + 1) * 512],
                                y_t[:, oc * 512:(oc + 1) * 512],
                            )
                    if not last_tile:
                        nc.scalar.dma_start(y[r0:r0 + 128, :], y_t[:])
    nc.finalize()
    return nc


_NC_CACHE = {}


def _get_nc(mm_dtype: str = MM_DTYPE):
    if mm_dtype not in _NC_CACHE:
        _NC_CACHE[mm_dtype] = _build_nc(mm_dtype)
    return _NC_CACHE[mm_dtype]


def _make_in_maps(x: np.ndarray, packed: np.ndarray, mm_dtype: str = MM_DTYPE):
    import ml_dtypes

    np_dt = _NP_DT.get(mm_dtype, np.dtype(ml_dtypes.bfloat16))
    xf = np.ascontiguousarray(x, dtype=np.float32).reshape(NCORES * R, K)
    pkt = np.ascontiguousarray(packed.T.astype(np.uint8))  # [128, 1024]
    in_maps = []
    for c in range(NCORES):
        xs = xf[c * R:(c + 1) * R]                       # [R, K]
        # k = 8j + b  ->  k' = b*128 + j ; [R,K]->[R,128,8]->[8,128,R]->[K,R]
        xp = np.ascontiguousarray(
            xs.reshape(R, 128, 8).transpose(2, 1, 0), dtype=np_dt
        ).reshape(K, R)
        # y = 2*(x @ bits.T) - rowsum(x): rowsum of the fp16-rounded shard
        srow = xp.astype(np.float64).sum(axis=0).astype(np.float32)  # [R]
        in_maps.append({"xp": xp, "pkt": pkt, "xs": srow})
    return in_maps


def kernel(x: np.ndarray, packed: np.ndarray) -> np.ndarray:
    x = np.asarray(x)
    packed = np.asarray(packed)
    assert x.shape == (2, 8192, K) and packed.shape == (O, K // 8)

    in_maps = _make_in_maps(x, packed)
    nc = _get_nc()
    res = run_bass_kernel_spmd(nc, in_maps, core_ids=list(range(NCORES)))
    out = np.concatenate([res.results[c]["y"] for c in range(NCORES)], axis=0)
    return out.reshape(2, 8192, O).astype(np.float32, copy=False)


# revision 12
# speedup vs baseline: 1.0484x; 1.0111x over previous
"""BitLinearPacked kernel for Trainium2 (8 NeuronCores, data-parallel).

y = x @ w.T where w = unpack_sign_bits(packed) in {-1, +1}.
  x: [2, 8192, 1024] fp32, packed: [1024, 128] int32 (8 sign bits / byte,
  MSB-first within each byte).

Strategy
--------
Data-parallel over the 16384 flattened rows of x: each of the 8 cores gets
2048 rows; the packed weight (128 KB as uint8) is replicated.

On-chip, matmul contracts over the partition dim, so both operands need
in_features (k) on partitions. We pre-transpose each x shard on the host
into [1024, 2048] — and, crucially, permute k as k' = b*128 + j (b = bit
index, j = byte index, k = 8j + b). Under that permutation the weight
plane for bit b is exactly ((packed.T >> (7-b)) & 1) * 2 - 1, computed
lane-local from one [128, 1024] uint8 tile of packed.T — no partition
remap / weight transpose needed on chip. The contraction is permutation-
invariant, so y is unchanged and comes out in natural [rows, out] layout.

Matmul dtype: fp16 streams 1 cycle/row through the PE (vs 2 for f32r and
4 for fp32); the weight (+/-1) is exact in any dtype, so the only error
is rounding x to fp16 (~11-bit significand) -> ~2e-4 absmax-relative.

Latency engineering (the steady-state MM stream is already at roofline):
- packed is shipped as uint8 (128 KB) and loaded via the scalar-engine
  HWDGE queue so it doesn't queue behind x loads on sync.
- window 0 runs bit-plane-major in two half-column phases: per plane, a
  small x DMA + half-column unpack + 4 matmuls, accumulating in 4 live
  PSUM banks per phase — the PE starts ~3 us after the preamble instead
  of waiting for the full unpack + full window DMA.
- dummy matmuls on a zeroed tile fill the initial DMA-wait so the PE's
  HAM clock gate is already at 2.4 GHz when the real stream starts.
- y stores go out on the scalar HWDGE queue, keeping sync free for x.
"""

import numpy as np

import concourse.bass as bass
import concourse.tile as tile
from concourse import bacc, mybir
from concourse.bass_utils import run_bass_kernel_spmd

NCORES = 8
R = 2048   # rows per core (16384 / 8)
K = 1024   # in_features
O = 1024   # out_features
RW = 512   # row window per x DMA

MM_DTYPE = "fp16"  # "fp16" | "bf16" | "f32r"
_DT = {
    "fp16": mybir.dt.float16,
    "bf16": mybir.dt.bfloat16,
    "f32r": mybir.dt.float32r,
}
_NP_DT = {"fp16": np.float16, "f32r": np.float32}
N_WARMUP_MM = 9


def _build_nc(mm_dtype: str = MM_DTYPE) -> bass.Bass:
    dt = _DT[mm_dtype]
    nc = bacc.Bacc("TRN2", target_bir_lowering=False, debug=False)
    xp = nc.declare_dram_parameter("xp", [K, R], dt, isOutput=False)
    pkt = nc.declare_dram_parameter("pkt", [128, O], mybir.dt.uint8, isOutput=False)
    xs_d = nc.declare_dram_parameter("xs", [R], mybir.dt.float32, isOutput=False)
    y = nc.declare_dram_parameter("y", [R, O], mybir.dt.float32, isOutput=True)

    # [K, R] -> [128 partitions, 8 k-chunks, R]
    xp_v = xp.rearrange("(c p) r -> p c r", p=128)
    xs_v = xs_d.rearrange("(t p) -> p t", p=128)
    n_oc = O // 512
    n_rt = RW // 128

    with tile.TileContext(nc) as tc:
        with (
            tc.tile_pool(name="wpool", bufs=1) as wpool,
            tc.tile_pool(name="pkpool", bufs=1) as pkpool,
            tc.tile_pool(name="bitpool", bufs=4) as bitpool,
            tc.tile_pool(name="xpool", bufs=2) as xpool,
            tc.tile_pool(name="ypool", bufs=3) as ypool,
            tc.tile_pool(name="pspool", bufs=7, space="PSUM") as pspool,
            tc.tile_pool(name="warmps", bufs=1, space="PSUM") as warmps,
        ):
            # --- PE warm-up: dummy matmuls on a zeroed tile, no data deps ---
            warm_sb = wpool.tile([128, 512], dt, name="warm_sb")
            nc.vector.memset(warm_sb[:], 0.0)
            warm_ps = warmps.tile([128, 512], mybir.dt.float32, name="warm_ps")
            for i in range(N_WARMUP_MM):
                nc.tensor.matmul(
                    warm_ps[:], lhsT=warm_sb[:, :128], rhs=warm_sb[:],
                    start=True, stop=True,
                )

            pk_t = pkpool.tile([128, O], mybir.dt.uint8)
            nc.scalar.dma_start(pk_t[:], pkt[:])
            xs_t = pkpool.tile([128, R // 128], mybir.dt.float32, name="xs_t")
            nc.scalar.dma_start(xs_t[:], xs_v[:])

            w_t = wpool.tile([128, 8, O], dt)
            x0_t = xpool.tile([128, 8, RW], dt, name="x0_t")


            # --- window 0: bit-plane-major, two half-column phases ---
            ps0 = [
                pspool.tile([128, 512], mybir.dt.float32, name=f"ps0_{i}", tag="ps")
                for i in range(n_rt * n_oc)
            ]
            for b in range(8):
                nc.sync.dma_start(x0_t[:, b:b + 1, :], xp_v[:, b:b + 1, 0:RW])
            for oc in range(n_oc):
                cs = slice(oc * 512, (oc + 1) * 512)
                for b in range(8):
                    bits = bitpool.tile(
                        [128, 512], mybir.dt.uint8, name=f"bits_{oc}_{b}", tag="bits"
                    )
                    nc.vector.tensor_scalar(
                        bits[:], pk_t[:, cs], 7 - b, 1,
                        mybir.AluOpType.logical_shift_right,
                        mybir.AluOpType.bitwise_and,
                    )
                    nc.scalar.copy(w_t[:, b, cs], bits[:])
                    for rt in range(n_rt):
                        nc.tensor.matmul(
                            ps0[rt * n_oc + oc][:],
                            lhsT=x0_t[:, b, rt * 128:(rt + 1) * 128],
                            rhs=w_t[:, b, cs],
                            start=(b == 0),
                            stop=(b == 7),
                        )
            for rt in range(n_rt):
                y_t = ypool.tile([128, O], mybir.dt.float32, name=f"y0_{rt}", tag="y_t")
                for oc in range(n_oc):
                    nc.vector.tensor_scalar(
                        y_t[:, oc * 512:(oc + 1) * 512], ps0[rt * n_oc + oc][:],
                        2.0, xs_t[:, rt:rt + 1],
                        mybir.AluOpType.mult, mybir.AluOpType.subtract,
                    )
                nc.scalar.dma_start(y[rt * 128:(rt + 1) * 128, :], y_t[:])

            # --- steady state: row-tile-major ---
            for rw in range(1, R // RW):
                x_t = xpool.tile([128, 8, RW], dt, name=f"x_t{rw}", tag="x_t")
                nc.sync.dma_start(x_t[:], xp_v[:, :, rw * RW:(rw + 1) * RW])
                for rt in range(n_rt):
                    r0 = rw * RW + rt * 128
                    y_t = ypool.tile(
                        [128, O], mybir.dt.float32, name=f"y_{rw}_{rt}", tag="y_t"
                    )
                    last_tile = (rw == R // RW - 1) and (rt == n_rt - 1)
                    for oc in range(n_oc):
                        ps = pspool.tile(
                            [128, 512], mybir.dt.float32,
                            name=f"ps_{rw}_{rt}_{oc}", tag="ps",
                        )
                        for b in range(8):
                            nc.tensor.matmul(
                                ps[:],
                                lhsT=x_t[:, b, rt * 128:(rt + 1) * 128],
                                rhs=w_t[:, b, oc * 512:(oc + 1) * 512],
                                start=(b == 0),
                                stop=(b == 7),
                            )
                        nc.vector.tensor_scalar(
                            y_t[:, oc * 512:(oc + 1) * 512], ps[:],
                            2.0, xs_t[:, rw * n_rt + rt:rw * n_rt + rt + 1],
                            mybir.AluOpType.mult, mybir.AluOpType.subtract,
                        )
                        if last_tile:
                            eng = nc.scalar if oc == 0 else nc.sync
                            eng.dma_start(
                                y[r0:r0 + 128, oc * 512:(oc + 1) * 512],
                                y_t[:, oc * 512:(oc + 1) * 512],
                            )
                    if not last_tile:
                        nc.scalar.dma_start(y[r0:r0 + 128, :], y_t[:])
    nc.finalize()
    return nc


_NC_CACHE = {}


def _get_nc(mm_dtype: str = MM_DTYPE):
    if mm_dtype not in _NC_CACHE:
        _NC_CACHE[mm_dtype] = _build_nc(mm_dtype)
    return _NC_CACHE[mm_dtype]


def _make_in_maps(x: np.ndarray, packed: np.ndarray, mm_dtype: str = MM_DTYPE):
    import ml_dtypes

    np_dt = _NP_DT.get(mm_dtype, np.dtype(ml_dtypes.bfloat16))
    xf = np.ascontiguousarray(x, dtype=np.float32).reshape(NCORES * R, K)
    pkt = np.ascontiguousarray(packed.T.astype(np.uint8))  # [128, 1024]
    in_maps = []
    for c in range(NCORES):
        xs = xf[c * R:(c + 1) * R]                       # [R, K]
        # k = 8j + b  ->  k' = b*128 + j ; [R,K]->[R,128,8]->[8,128,R]->[K,R]
        xp = np.ascontiguousarray(
            xs.reshape(R, 128, 8).transpose(2, 1, 0), dtype=np_dt
        ).reshape(K, R)
        # y = 2*(x @ bits.T) - rowsum(x): rowsum of the fp16-rounded shard
        srow = xp.astype(np.float64).sum(axis=0).astype(np.float32)  # [R]
        in_maps.append({"xp": xp, "pkt": pkt, "xs": srow})
    return in_maps


def kernel(x: np.ndarray, packed: np.ndarray) -> np.ndarray:
    x = np.asarray(x)
    packed = np.asarray(packed)
    assert x.shape == (2, 8192, K) and packed.shape == (O, K // 8)

    in_maps = _make_in_maps(x, packed)
    nc = _get_nc()
    res = run_bass_kernel_spmd(nc, in_maps, core_ids=list(range(NCORES)))
    out = np.concatenate([res.results[c]["y"] for c in range(NCORES)], axis=0)
    return out.reshape(2, 8192, O).astype(np.float32, copy=False)
